# revision 38
# baseline (speedup 1.0000x reference)
"""MoE layer (8 experts, top-2) as an expert-parallel Trainium2 Bass kernel.

Strategy:
  - Host: gating matmul (tiny), top-2 routing, gather tokens per expert.
  - Device (8 NeuronCores, SPMD, one expert per core): FFN over the expert's
    tokens — h = relu(x @ W1 + b1); y = (h @ W2) * combine_weight — in bf16
    with fp32 PSUM accumulation.
  - Host: scatter-add the two expert contributions per token, add b2 term.

Layouts (device side, per core):
  xT : [128, 8*C]    bf16   xT[p, ko*C + c]   = x_tok[c, ko*128+p]
  w1 : [128, 8*4096] bf16   w1[p, ko*F + f]   = W1[ko*128+p, f]
  w2 : [128, 32*1024]bf16   w2[p, ko*D + d]   = W2[ko*128+p, d]
  b1 : [128, 32]     f32    b1[p, ft]         = b1_vec[ft*128+p]
  cw : [128, C/128]  f32    cw[p, o]          = combine_weight[o*128+p]
  y  : [128, (C/128)*1024] f32 (out)  y[p, o*D+d] = y_tok[o*128+p, d]
"""

import os

import numpy as np
import ml_dtypes

D_MODEL = 1024
D_FF = 4096
N_EXPERTS = 8
TOP_K = 2
B, S = 4, 2048
T = B * S
P = 128
KO1 = D_MODEL // P   # 8  k-subtiles for matmul1
KO2 = D_FF // P      # 32 k-subtiles for matmul2
N_CORES = 8

BF16 = ml_dtypes.bfloat16

# Compiled-module cache keyed by padded capacity C.
_NC_CACHE = {}
LAST_RESULTS = None  # BassKernelResults of the most recent run (for test.py)
LAST_IN_MAPS = None  # per-core input maps of the most recent run
LAST_C = None


def _blocks_for(C):
    """Split C tokens into matmul token-blocks.

    First block is 384 so the first matmul group's input DMAs are small
    (fast time-to-first-matmul); keep every block >= ~192 so matmul free
    dims stay large enough to hide LDWEIGHTS. Only the last block may be
    a non-multiple of 128 (handled with a partial final m-tile)."""
    frac = C % P
    blocks = []
    rem = C - frac
    if rem >= 896:
        blocks.append(384)
        rem -= 384
    while rem >= 768:
        blocks.append(512)
        rem -= 512
    if rem > 512:
        blocks.extend([rem - 256, 256])
    elif rem:
        blocks.append(rem)
    if frac:
        # Fold the sub-128 remainder into the LAST block only: every other
        # block offset must stay a multiple of 128.
        if not blocks:
            blocks = [frac]
        elif blocks[-1] + frac <= 512:
            blocks[-1] += frac
        else:
            blocks[-1:] = [256, blocks[-1] + frac - 256]
    return blocks


def _build_nc(C, Cact=None, reps=1):
    import concourse.bass as bass  # noqa: F401
    import concourse.tile as tile
    from concourse import bacc, mybir
    from contextlib import ExitStack

    if Cact is None:
        Cact = C
    OUTERS = C // P

    nc = bacc.Bacc("TRN2", target_bir_lowering=False, debug=False,
                   num_devices=N_CORES)

    xT = nc.dram_tensor("xT", [P, KO1 * C], mybir.dt.bfloat16,
                        kind="ExternalInput")
    w1 = nc.dram_tensor("w1", [P, KO1 * D_FF], mybir.dt.bfloat16,
                        kind="ExternalInput")
    w2 = nc.dram_tensor("w2", [P, KO2 * D_MODEL], mybir.dt.bfloat16,
                        kind="ExternalInput")
    b1 = nc.dram_tensor("b1", [P, KO2], mybir.dt.float32,
                        kind="ExternalInput")
    cw = nc.dram_tensor("cw", [P, OUTERS], mybir.dt.float32,
                        kind="ExternalInput")
    y = nc.dram_tensor("y", [P, OUTERS * D_MODEL], mybir.dt.float32,
                       kind="ExternalOutput")

    xT_ap = xT.ap().rearrange("p (ko c) -> p ko c", ko=KO1)
    w1_ap = w1.ap().rearrange("p (ko f) -> p ko f", ko=KO1)
    w2_ap = w2.ap().rearrange("p (ko d) -> p ko d", ko=KO2)
    y_ap = y.ap()

    # Schedule only the actual max expert load; the layout stays padded to
    # C (multiple of 128) but the last block / final m-tile are partial.
    blocks = _blocks_for(Cact)
    chunk_offs = []
    off = 0
    for TB in blocks:
        chunk_offs.append((off, TB))
        off += TB
    FE = 512                # W1 loaded in eighths of the f axis
    FT_E = FE // P          # 4 f-tiles per eighth

    with tile.TileContext(nc) as tc, ExitStack() as ctx:
        wpool = ctx.enter_context(tc.tile_pool(name="wpool", bufs=1))
        xpool = ctx.enter_context(tc.tile_pool(name="xpool", bufs=3))
        hpool = ctx.enter_context(tc.tile_pool(name="hpool", bufs=1))
        ypool = ctx.enter_context(tc.tile_pool(name="ypool", bufs=4))
        ps1 = ctx.enter_context(tc.tile_pool(name="ps1", bufs=3, space="PSUM"))
        ps2 = ctx.enter_context(tc.tile_pool(name="ps2", bufs=5, space="PSUM"))

        # (reps>1 repeats the whole body back-to-back; timing-only)
        for _rep in range(reps):
            # Fully resident weights: W1 8MB + W2 8MB bf16.
            W1s = wpool.tile([P, KO1, D_FF], mybir.dt.bfloat16, tag="W1s",
                             name="W1s")
            W2s = wpool.tile([P, KO2, D_MODEL], mybir.dt.bfloat16, tag="W2s",
                             name="W2s")
            b1s = wpool.tile([P, KO2], mybir.dt.float32, tag="b1s",
                             name="b1s")
            cws = wpool.tile([P, OUTERS], mybir.dt.float32, tag="cws",
                             name="cws")

            # Prologue DMAs, all issued from the Sync queue (engine-issued
            # DMAs from gpsimd/scalar land ~10us late -- slow queue type).
            # Descriptor issue costs ~0.6us serial and transfers share
            # ~358 GB/s, so critical-path data goes first in few, small
            # descriptors, in consumption order: x(b0) + W1 f 0:128 (first
            # matmul group), b1 (first relu), rest of W1's first slab,
            # x(b1), then W1 slabs interleaved with W2 chunks (mm2 of b0
            # needs W2 from ~45us; mm1 of b1 needs all W1 from ~45us).
            xts = {}
            c0_0, TB_0 = chunk_offs[0]
            xts[0] = xpool.tile([P, KO1, 512], mybir.dt.bfloat16, tag="x",
                                name="xt")
            nc.sync.dma_start(xts[0][:, 0:4, :TB_0],
                              xT_ap[:, 0:4, c0_0:c0_0 + TB_0])
            nc.sync.dma_start(W1s[:, 0:4, 0:P], w1_ap[:, 0:4, 0:P])
            nc.sync.dma_start(xts[0][:, 4:8, :TB_0],
                              xT_ap[:, 4:8, c0_0:c0_0 + TB_0])
            nc.sync.dma_start(W1s[:, 4:8, 0:P], w1_ap[:, 4:8, 0:P])
            nc.sync.dma_start(b1s[:], b1.ap())
            nc.sync.dma_start(W1s[:, :, P:FE], w1_ap[:, :, P:FE])
            if len(chunk_offs) > 1:
                c0_1, TB_1 = chunk_offs[1]
                xts[1] = xpool.tile([P, KO1, 512], mybir.dt.bfloat16,
                                    tag="x", name="xt")
                nc.sync.dma_start(xts[1][:, :, :TB_1],
                                  xT_ap[:, :, c0_1:c0_1 + TB_1])
            for q in range(1, 4):
                nc.sync.dma_start(W1s[:, :, q * FE:(q + 1) * FE],
                                  w1_ap[:, :, q * FE:(q + 1) * FE])
            nc.sync.dma_start(W2s[:, 0:2, :], w2_ap[:, 0:2, :])
            for q in range(4, D_FF // FE):
                nc.sync.dma_start(W1s[:, :, q * FE:(q + 1) * FE],
                                  w1_ap[:, :, q * FE:(q + 1) * FE])
                k2 = 2 + (q - 4) * 2
                nc.sync.dma_start(W2s[:, k2:k2 + 2, :],
                                  w2_ap[:, k2:k2 + 2, :])
            nc.sync.dma_start(cws[:], cw.ap())
            for k2 in range(10, KO2, 2):
                nc.sync.dma_start(W2s[:, k2:k2 + 2, :],
                                  w2_ap[:, k2:k2 + 2, :])

            for ci, (c0, TB) in enumerate(chunk_offs):
                xt = xts.pop(ci)
                # Prefetch x for block ci+2 (block ci+1 already in flight).
                if ci + 2 < len(chunk_offs):
                    c0n, TBn = chunk_offs[ci + 2]
                    xts[ci + 2] = xpool.tile([P, KO1, 512],
                                             mybir.dt.bfloat16,
                                             tag="x", name="xt")
                    nc.sync.dma_start(xts[ci + 2][:, :, :TBn],
                                      xT_ap[:, :, c0n:c0n + TBn])

                # mm1: h[f, tok] = relu(x @ W1 + b1), kept in SBUF.
                # Only computed for the Cact real tokens: the pad columns of
                # ht keep stale (finite) values from the previous block; mm2
                # multiplies those rows by cw=0 into the never-read pad
                # region, same as the zero-padded result.
                TBa = min(TB, Cact - c0)
                ht = hpool.tile([P, KO2, 512], mybir.dt.bfloat16, tag="h",
                                name="ht")
                for ft in range(KO2):
                    pt = ps1.tile([P, 512], mybir.dt.float32, tag="ps1",
                                  name="pt")
                    for ko in range(KO1):
                        nc.tensor.matmul(
                            pt[:, :TBa],
                            W1s[:, ko, ft * P:(ft + 1) * P],
                            xt[:, ko, :TBa],
                            start=(ko == 0),
                            stop=(ko == KO1 - 1),
                        )
                    nc.scalar.activation(
                        ht[:, ft, :TBa], pt[:, :TBa],
                        mybir.ActivationFunctionType.Relu,
                        bias=b1s[:, ft:ft + 1],
                    )

                # mm2: y[tok, d] = (h @ W2) * cw, straight out of SBUF h.
                # The final m-tile may cover fewer than 128 tokens.
                outer0 = c0 // P
                for ms in range((TB + P - 1) // P):
                    msz = min(P, TB - ms * P)
                    pt2a = ps2.tile([P, 512], mybir.dt.float32,
                                    tag="ps2", name="pt2a")
                    pt2b = ps2.tile([P, 512], mybir.dt.float32,
                                    tag="ps2", name="pt2b")
                    for ko in range(KO2):
                        lhsT = ht[:, ko, ms * P:ms * P + msz]
                        nc.tensor.matmul(
                            pt2a[:msz, :], lhsT, W2s[:, ko, 0:512],
                            start=(ko == 0), stop=(ko == KO2 - 1),
                        )
                        nc.tensor.matmul(
                            pt2b[:msz, :], lhsT, W2s[:, ko, 512:1024],
                            start=(ko == 0), stop=(ko == KO2 - 1),
                        )
                    outer = outer0 + ms
                    for nt, pt2 in enumerate((pt2a, pt2b)):
                        yt = ypool.tile([P, 512], mybir.dt.float32,
                                        tag="y", name="yt")
                        # Split the combine-weight multiply across the scalar
                        # and vector engines so the two halves of an m-tile
                        # drain PSUM in parallel (matters at the kernel tail).
                        # Full 128 partitions even when the matmul was
                        # partial: a partial-partition DRAM DMA takes a ~17us
                        # slow path, and rows >= msz multiply stale-but-
                        # finite PSUM by cw=0 into the never-read pad region.
                        if nt == 0:
                            nc.scalar.mul(yt[:], pt2[:],
                                          cws[:, outer:outer + 1])
                        else:
                            nc.vector.tensor_scalar_mul(
                                yt[:], pt2[:], cws[:, outer:outer + 1])
                        nc.sync.dma_start(
                            y_ap[:, outer * D_MODEL + nt * 512:
                                 outer * D_MODEL + (nt + 1) * 512],
                            yt[:],
                        )

    nc.compile()
    return nc


def _route(x_flat, Wg, bg):
    logits = x_flat.astype(np.float32) @ Wg.astype(np.float32) + bg
    idx = np.argsort(-logits, axis=1, kind="stable")[:, :TOP_K]
    gates = np.take_along_axis(logits, idx, axis=1)  # [T, 2] descending
    e1 = np.exp(gates[:, 1] - gates[:, 0])
    denom = 1.0 + e1
    w = np.stack([1.0 / denom, e1 / denom], axis=1).astype(np.float32)
    return idx.astype(np.int32), w


def kernel(x, Wg, bg, W1, b1, W2, b2):
    global LAST_RESULTS
    x = np.asarray(x, dtype=np.float32)
    Wg = np.asarray(Wg, dtype=np.float32)
    bg = np.asarray(bg, dtype=np.float32)
    W1 = np.asarray(W1, dtype=np.float32)
    b1 = np.asarray(b1, dtype=np.float32)
    W2 = np.asarray(W2, dtype=np.float32)
    b2 = np.asarray(b2, dtype=np.float32)

    x_flat = x.reshape(T, D_MODEL)
    idx, w = _route(x_flat, Wg, bg)

    # Per-expert token lists + slot map (position of each (token, k) pair
    # inside its expert's gathered block).
    tok_lists = []
    counts = []
    slot = np.empty((T, TOP_K), dtype=np.int64)
    for e in range(N_EXPERTS):
        mask = (idx[:, 0] == e) | (idx[:, 1] == e)
        tok = np.nonzero(mask)[0]
        tok_lists.append(tok)
        counts.append(len(tok))
        which = (idx[tok, 1] == e).astype(np.int64)  # 0 if k=0 slot, else 1
        slot[tok, which] = np.arange(len(tok))

    Cact = max(counts)
    C = ((Cact + P - 1) // P) * P

    if (C, Cact) not in _NC_CACHE:
        _NC_CACHE[(C, Cact)] = _build_nc(C, Cact)
    nc = _NC_CACHE[(C, Cact)]

    # Build per-core input maps.
    in_maps = []
    for e in range(N_EXPERTS):
        tok = tok_lists[e]
        n = len(tok)
        xg = np.zeros((C, D_MODEL), dtype=np.float32)
        xg[:n] = x_flat[tok]
        wt = np.zeros((C,), dtype=np.float32)
        we = np.where(idx[tok, 0] == e, w[tok, 0], w[tok, 1])
        wt[:n] = we

        xT_dev = np.ascontiguousarray(
            xg.reshape(C, KO1, P).transpose(2, 1, 0)
        ).reshape(P, KO1 * C).astype(BF16)
        w1_dev = np.ascontiguousarray(
            W1[e].reshape(KO1, P, D_FF).transpose(1, 0, 2)
        ).reshape(P, KO1 * D_FF).astype(BF16)
        w2_dev = np.ascontiguousarray(
            W2[e].reshape(KO2, P, D_MODEL).transpose(1, 0, 2)
        ).reshape(P, KO2 * D_MODEL).astype(BF16)
        b1_dev = np.ascontiguousarray(b1[e].reshape(KO2, P).T)
        cw_dev = np.ascontiguousarray(wt.reshape(C // P, P).T)

        in_maps.append({
            "xT": xT_dev,
            "w1": w1_dev,
            "w2": w2_dev,
            "b1": b1_dev.astype(np.float32),
            "cw": cw_dev.astype(np.float32),
        })

    from concourse.bass_utils import run_bass_kernel_spmd

    global LAST_IN_MAPS, LAST_C
    LAST_IN_MAPS = in_maps
    LAST_C = C

    trace = os.environ.get("MOE_KERNEL_TRACE", "0") == "1"
    res = run_bass_kernel_spmd(
        nc, in_maps, core_ids=list(range(N_CORES)),
        trace=trace, trace_cores=[0] if trace else None,
    )
    LAST_RESULTS = res

    # Unpack per-core outputs: y_dev [P, (C/P)*D] -> [C, D]
    Yall = np.empty((N_EXPERTS, C, D_MODEL), dtype=np.float32)
    for e in range(N_EXPERTS):
        y_dev = res.results[e]["y"]
        Yall[e] = (
            y_dev.reshape(P, C // P, D_MODEL)
            .transpose(1, 0, 2)
            .reshape(C, D_MODEL)
        )

    tok_all = np.arange(T)
    out_flat = (
        Yall[idx[:, 0], slot[tok_all, 0]] + Yall[idx[:, 1], slot[tok_all, 1]]
    )

    if np.any(b2):
        out_flat += w[:, 0:1] * b2[idx[:, 0]] + w[:, 1:2] * b2[idx[:, 1]]

    return out_flat.reshape(B, S, D_MODEL).astype(np.float32)

